# revision 19
# baseline (speedup 1.0000x reference)
"""GATv2 layer (100k nodes, 800k edges + self-loops, 8 heads x 16 dim) on 8 TRN2
cores — v2.

Destination nodes are partitioned across the 8 cores (12.5k each).  Real edges
are bucketed per (dst-group-of-128, src-quartile) cell with a shared SPMD
schedule; self-loops run as a dense per-span pseudo-segment with identity
selection (they open each group's PSUM accumulators).

Key structure (v2):
- Gather descriptor generation is the scarce resource (~8 ns/row of Q7 time),
  so gathers are issued one-per-segment and round-robined over 4 SWDGE queues:
  each queue's descriptors are generated by a different Q7 core pair.
- One-hot selection matrices (sel = emission lhsT, selT = xr-selection lhsT)
  are precomputed on the host in fp8 (0/1 exact) and streamed from HBM —
  no on-device is_equal / transpose / PSUM copies.
- m = xr[dst] + xl[src] is accumulated on the PE: selT-matmul from the
  span-resident xr table plus an identity-matmul of the gathered xl rows.
  LeakyReLU runs on the scalar engine straight out of PSUM.
- Features use a d-major permutation (col d*8+h holds head h, dim d), baked
  into W_l/W_r/att/residual on the host: the per-head exp() broadcast multiply
  and the attention-dot reduction (a pure halving tree) then run at the DVE's
  2x bf16 rate.  The final LayerNorm op un-permutes via its output AP.
"""

import math

import numpy as np
import ml_dtypes

P = 128
H, D = 8, 16
IN = 128
OUT = 128
NEG_SLOPE = 0.2
LN_EPS = 1e-5
DEN_EPS = 1e-16

N_CORES = 8
NQ = 4            # src quartiles (int16 gather index range)
G_SPAN = 8        # dst groups per span (3 PSUM emission banks per span)
N_QUEUES = 4      # SWDGE queues for gather DGE parallelism

BF16 = ml_dtypes.bfloat16
FP8 = ml_dtypes.float8_e4m3

# permutation: permuted column d*8+h holds original column h*16+d
PERM = np.array([h * D + d for d in range(D) for h in range(H)], dtype=np.int64)


# ---------------------------------------------------------------------------
# CPU preprocessing: cell bucketing + static SPMD schedule + one-hot blobs
# ---------------------------------------------------------------------------

def _preprocess(edge_index: np.ndarray, n_nodes: int, n_cores: int):
    src = edge_index[0].astype(np.int64)
    dst = edge_index[1].astype(np.int64)

    assert n_nodes % n_cores == 0
    per = n_nodes // n_cores
    qs = n_nodes // NQ
    n_groups = math.ceil(per / P)
    n_spans = math.ceil(n_groups / G_SPAN)

    core = dst // per
    g_loc = (dst - core * per) // P
    quart = src // qs
    span = g_loc // G_SPAN

    key = (((core * n_spans + span) * NQ + quart) * n_groups + g_loc) * np.int64(
        n_nodes
    ) + src
    order = np.argsort(key, kind="stable")
    src = src[order]
    dst = dst[order]
    core = core[order]
    g_loc = g_loc[order]
    quart = quart[order]

    # shared cell caps (32-multiples, >=128 so subtiles span <=2 groups)
    cell_key = (core * n_groups + g_loc) * NQ + quart
    cnt = np.bincount(cell_key, minlength=n_cores * n_groups * NQ).reshape(
        n_cores, n_groups, NQ
    )
    cap = np.maximum(P, ((cnt.max(axis=0) + 31) // 32) * 32)  # [n_groups, NQ]

    seg_sub = np.zeros((n_spans, NQ), dtype=np.int64)
    cell_off = np.zeros((n_groups, NQ), dtype=np.int64)
    emits = {}        # (s,q) -> [(j, g)] in emission order
    for s in range(n_spans):
        gs = list(range(s * G_SPAN, min((s + 1) * G_SPAN, n_groups)))
        for q in range(NQ):
            off = 0
            bounds = []
            for g in gs:
                cell_off[g, q] = off
                bounds.append((off, g))
                off += cap[g, q]
            n_sub = (off + P - 1) // P
            seg_sub[s, q] = n_sub
            em = []
            for j in range(n_sub):
                lo, hi = j * P, min((j + 1) * P, off)
                cells = [g for (st, g) in bounds
                         if st < hi and st + cap[g, q] > lo]
                assert 1 <= len(cells) <= 2, (s, q, j, cells)
                for g in cells:
                    em.append((j, g))
            emits[(s, q)] = em
    S_max = int(seg_sub.max())
    assert S_max * P <= 2944, "gather exceeds SWDGE ring"

    # last touch per PSUM bank (3 groups each) across the span's real-edge
    # emission streams; None if the bank only sees its self-segment opener
    bank_stop = {}
    for s in range(n_spans):
        gs = list(range(s * G_SPAN, min((s + 1) * G_SPAN, n_groups)))
        n_banks = math.ceil(len(gs) / 3)
        for b in range(n_banks):
            bank_stop[(s, b)] = None
        for q in range(NQ):
            for i, (j, g) in enumerate(emits[(s, q)]):
                bank_stop[(s, (g - gs[0]) // 3)] = (q, i)

    # column layouts
    seg_idx_off = np.zeros((n_spans, NQ), dtype=np.int64)   # idx cols (8S each)
    seg_sel_off = np.zeros((n_spans, NQ), dtype=np.int64)   # sel cols (128*em)
    seg_em = np.zeros((n_spans, NQ), dtype=np.int64)
    c_idx = 0
    c_sel = 0
    for s in range(n_spans):
        for q in range(NQ):
            seg_idx_off[s, q] = c_idx
            c_idx += 8 * seg_sub[s, q]
            seg_sel_off[s, q] = c_sel
            seg_em[s, q] = len(emits[(s, q)])
            c_sel += P * seg_em[s, q]

    # per-segment slot base in the flat slot vector
    seg_slot_off = np.zeros((n_spans, NQ), dtype=np.int64)
    t = 0
    for s in range(n_spans):
        for q in range(NQ):
            seg_slot_off[s, q] = t
            t += seg_sub[s, q] * P
    total_slots = t

    em_max = int(seg_em.max())

    idx_arrays = []
    sel_arrays = []
    selT_arrays = []
    for c in range(n_cores):
        m = core == c
        e_src = src[m]
        e_dst = dst[m]
        e_g = g_loc[m]
        e_q = quart[m]
        e_span = e_g // G_SPAN
        ck = (e_span * NQ + e_q) * n_groups + e_g
        changes = np.ones(len(ck), dtype=bool)
        changes[1:] = ck[1:] != ck[:-1]
        starts = np.flatnonzero(changes)
        rank = np.arange(len(ck)) - np.repeat(starts, np.diff(
            np.append(starts, len(ck))))
        slot = (seg_slot_off[e_span, e_q] + cell_off[e_g, e_q] + rank)

        xl_idx = np.zeros(total_slots, dtype=np.int16)
        dloc = np.full(total_slots, -1, dtype=np.int64)   # dst local row
        xl_idx[slot] = (e_src - e_q * qs).astype(np.int16)
        dloc[slot] = e_dst - c * per

        packed = np.zeros((P, c_idx), dtype=np.int16)
        sel8 = np.zeros((P, c_sel), dtype=FP8)
        selT8 = np.zeros((P, c_sel), dtype=FP8)
        for s in range(n_spans):
            for q in range(NQ):
                S = int(seg_sub[s, q])
                if S == 0:
                    continue
                o = int(seg_slot_off[s, q])
                n = S * P
                co = int(seg_idx_off[s, q])
                packed[:, co : co + 8 * S] = np.tile(
                    xl_idx[o : o + n].reshape(-1, 16).T, (8, 1)
                )
                so = int(seg_sel_off[s, q])
                dl = dloc[o : o + n].reshape(S, P)    # [subtile, slot]
                for i, (j, g) in enumerate(emits[(s, q)]):
                    hot = dl[j] - g * P               # [P] values or <0
                    ok = (hot >= 0) & (hot < P)
                    rows = np.flatnonzero(ok)
                    one = np.zeros((P, P), dtype=FP8)
                    one[rows, hot[rows]] = 1.0
                    sel8[:, so + i * P : so + (i + 1) * P] = one
                    selT8[:, so + i * P : so + (i + 1) * P] = one.T
        idx_arrays.append(packed)
        sel_arrays.append(sel8)
        selT_arrays.append(selT8)

    sched = {
        "n_groups": n_groups,
        "n_spans": n_spans,
        "seg_sub": seg_sub,
        "seg_idx_off": seg_idx_off,
        "seg_sel_off": seg_sel_off,
        "seg_em": seg_em,
        "emits": emits,
        "bank_stop": bank_stop,
        "c_idx": c_idx,
        "c_sel": c_sel,
        "S_max": S_max,
        "em_max": em_max,
    }
    return sched, idx_arrays, sel_arrays, selT_arrays, per


# ---------------------------------------------------------------------------
# Bass program (shared by all cores)
# ---------------------------------------------------------------------------

def _build_program(n_nodes, per, sched):
    from contextlib import ExitStack

    from concourse import bass, mybir
    from concourse import tile as tile_mod
    from concourse.bacc import Bacc

    f32 = mybir.dt.float32
    bf16 = mybir.dt.bfloat16
    fp8 = mybir.dt.float8e4
    i16 = mybir.dt.int16
    Alu = mybir.AluOpType
    Act = mybir.ActivationFunctionType

    qs = n_nodes // NQ
    n_groups = sched["n_groups"]
    n_spans = sched["n_spans"]
    seg_sub = sched["seg_sub"]
    seg_idx_off = sched["seg_idx_off"]
    seg_sel_off = sched["seg_sel_off"]
    emits = sched["emits"]
    bank_stop = sched["bank_stop"]
    c_idx = sched["c_idx"]
    c_sel = sched["c_sel"]
    S_max = sched["S_max"]
    em_max = sched["em_max"]
    own_pad = n_groups * P
    last_rows = per - (n_groups - 1) * P

    n_blk_q = math.ceil((n_nodes // NQ) / 1024)   # phase-1 blocks per quartile
    q_pad = n_blk_q * 1024
    x_pad = (NQ - 1) * qs + q_pad
    n_blk_own = math.ceil(own_pad / 1024)
    own_x_pad = n_blk_own * 1024

    nc = Bacc(dynamic_dma_scratch_size=32768, num_swdge_queues=N_QUEUES)

    x16t = nc.declare_dram_parameter("x16t", [IN, x_pad], bf16,
                                     isOutput=False)
    xo16t = nc.declare_dram_parameter("xo16t", [IN, own_x_pad], bf16,
                                      isOutput=False)
    wl_d = nc.declare_dram_parameter("wl16", [IN, OUT], bf16, isOutput=False)
    wr_d = nc.declare_dram_parameter("wr16", [IN, OUT], bf16, isOutput=False)
    att_d = nc.declare_dram_parameter("att16", [P, OUT], bf16, isOutput=False)
    id8_d = nc.declare_dram_parameter("ident8", [P, P], fp8, isOutput=False)
    idx_d = nc.declare_dram_parameter("idx", [P, c_idx], i16, isOutput=False)
    sel_d = nc.declare_dram_parameter("sel8", [P, c_sel], fp8, isOutput=False)
    selT_d = nc.declare_dram_parameter("selT8", [P, c_sel], fp8,
                                       isOutput=False)
    xres_d = nc.declare_dram_parameter("xres16", [own_pad, IN], bf16,
                                       isOutput=False)
    out_own = nc.declare_dram_parameter("out_own", [own_pad, OUT], f32,
                                        isOutput=True)

    xl_tab_q = [nc.dram_tensor(f"xl_tab_q{q}", [q_pad, OUT], bf16)
                for q in range(NQ)]
    xlo_tab = nc.dram_tensor("xlo_tab", [own_pad, OUT], bf16)

    with tile_mod.TileContext(nc) as tc, ExitStack() as ctx:
        const = ctx.enter_context(tc.tile_pool(name="const", bufs=1))
        wl_s = const.tile([IN, OUT], bf16)
        wr_s = const.tile([IN, OUT], bf16)
        att_s = const.tile([P, OUT], bf16)
        id8_s = const.tile([P, P], fp8)
        xrs_all = const.tile([P, n_groups, OUT], bf16)   # own xr, resident
        nc.sync.dma_start(out=wl_s[:], in_=wl_d[:])
        nc.sync.dma_start(out=wr_s[:], in_=wr_d[:])
        nc.sync.dma_start(out=att_s[:], in_=att_d[:])
        nc.sync.dma_start(out=id8_s[:], in_=id8_d[:])

        # PE warmup: observe DMA-loaded weights once.
        with tc.tile_pool(name="warm", bufs=1, space="PSUM") as warm:
            warm_p = warm.tile([P, OUT], f32)
            nc.tensor.matmul(out=warm_p[:], lhsT=wl_s[:], rhs=wl_s[:],
                             start=True, stop=True)
            nc.tensor.matmul(out=warm_p[:], lhsT=wr_s[:], rhs=wr_s[:],
                             start=True, stop=True)
            nc.tensor.matmul(out=warm_p[:], lhsT=id8_s[:], rhs=wl_s[:],
                             start=True, stop=True)

        # ------------------------------------------------------------------
        # Phase 1a: own rows -> resident xr / xl tables (no HBM roundtrip)
        # ------------------------------------------------------------------
        with tc.tile_pool(name="p1o", bufs=3) as p1o, \
             tc.tile_pool(name="p1ops", bufs=2, space="PSUM") as p1ops:
            for b in range(n_blk_own):
                r0 = b * 1024
                xt = p1o.tile([P, 1024], bf16, tag="xt")
                nc.sync.dma_start(out=xt[:], in_=xo16t[:, r0 : r0 + 1024])
                g0 = r0 // P
                n_j = min(8, n_groups - g0)
                # xr: into the resident SBUF table
                o_p = p1ops.tile([P, 1024], f32, tag="opr")
                for j in range(n_j):
                    nc.tensor.matmul(
                        out=o_p[:, j * P : (j + 1) * P],
                        lhsT=xt[:, j * P : (j + 1) * P],
                        rhs=wr_s[:],
                        start=True,
                        stop=True,
                    )
                nc.scalar.copy(
                    out=xrs_all[:, g0 : g0 + n_j, :].rearrange(
                        "p j f -> p (j f)"
                    ),
                    in_=o_p[:, : n_j * P],
                )
                # xl of own rows: staged to DRAM (per-span self segments)
                o_p2 = p1ops.tile([P, 1024], f32, tag="opl")
                for j in range(n_j):
                    nc.tensor.matmul(
                        out=o_p2[:, j * P : (j + 1) * P],
                        lhsT=xt[:, j * P : (j + 1) * P],
                        rhs=wl_s[:],
                        start=True,
                        stop=True,
                    )
                stg = p1o.tile([P, 8, OUT], bf16, tag="stg")
                nc.scalar.copy(
                    out=stg[:, :n_j, :].rearrange("p j f -> p (j f)"),
                    in_=o_p2[:, : n_j * P],
                )
                nc.scalar.dma_start(
                    out=xlo_tab[r0 : r0 + n_j * P, :].rearrange(
                        "(j p) f -> p j f", p=P
                    ),
                    in_=stg[:, :n_j, :],
                )

        # ------------------------------------------------------------------
        # Phase 1b: xl projection tables (gather sources), one per quartile
        # so phase-2 segments for quartile q only wait on q's table
        # ------------------------------------------------------------------
        with tc.tile_pool(name="p1", bufs=6) as p1, \
             tc.tile_pool(name="p1ps", bufs=4, space="PSUM") as p1ps:
            for q in range(NQ):
                for b in range(n_blk_q):
                    r0 = q * qs + b * 1024
                    xt = p1.tile([P, 1024], bf16, tag="xt")
                    nc.sync.dma_start(out=xt[:], in_=x16t[:, r0 : r0 + 1024])
                    o_p = p1ps.tile([P, 1024], f32, tag="op")
                    for j in range(8):
                        nc.tensor.matmul(
                            out=o_p[:, j * P : (j + 1) * P],
                            lhsT=xt[:, j * P : (j + 1) * P],
                            rhs=wl_s[:],
                            start=True,
                            stop=True,
                        )
                    stg = p1.tile([P, 8, OUT], bf16, tag="stg")
                    if b % 2 == 0:
                        nc.scalar.copy(
                            out=stg[:].rearrange("p j f -> p (j f)"),
                            in_=o_p[:],
                        )
                    else:
                        nc.vector.tensor_copy(
                            out=stg[:].rearrange("p j f -> p (j f)"),
                            in_=o_p[:],
                        )
                    nc.sync.dma_start(
                        out=xl_tab_q[q][b * 1024 : (b + 1) * 1024, :].rearrange(
                            "(j p) f -> p j f", p=P
                        ),
                        in_=stg[:],
                    )

        # ------------------------------------------------------------------
        # Phase 2
        # ------------------------------------------------------------------
        with tc.tile_pool(name="p2", bufs=2) as p2, \
             tc.tile_pool(name="gps", bufs=2, space="PSUM") as gps, \
             tc.tile_pool(name="xps", bufs=2, space="PSUM") as xps, \
             tc.tile_pool(name="ep", bufs=2) as ep:

            att_bc = att_s[:][:, None, :]
            gather_rr = [0]

            def seg_compute(S, xl_t, m_src, wt):
                """lrelu -> att-dot tree -> exp -> weighted values.

                m_src(c0, c1) yields the PSUM AP holding m for subtiles
                [c0, c1); xl_t is the SBUF bf16 xl rows tile [P, S, OUT].
                Returns the wt tile filled in [:, :S, :136].
                """
                t = p2.tile([P, S_max, OUT], bf16, tag="t")
                for c0 in range(0, S, 4):
                    c1 = min(c0 + 4, S)
                    nc.scalar.activation(
                        out=t[:, c0:c1, :], in_=m_src(c0, c1),
                        func=Act.Prelu, alpha=NEG_SLOPE,
                    )
                u = p2.tile([P, S_max, OUT], bf16, tag="u")
                nc.vector.tensor_tensor(
                    out=u[:, :S, :], in0=t[:, :S, :],
                    in1=att_bc.to_broadcast((P, S, OUT)), op=Alu.mult,
                )
                # halving tree over d (d-major layout: col = d*8 + h)
                r64 = p2.tile([P, S_max, 64], bf16, tag="r64")
                nc.vector.tensor_tensor(out=r64[:, :S, :], in0=u[:, :S, 0:64],
                                        in1=u[:, :S, 64:128], op=Alu.add)
                r32 = p2.tile([P, S_max, 32], bf16, tag="r32")
                nc.vector.tensor_tensor(out=r32[:, :S, :], in0=r64[:, :S, 0:32],
                                        in1=r64[:, :S, 32:64], op=Alu.add)
                r16 = p2.tile([P, S_max, 16], bf16, tag="r16")
                nc.vector.tensor_tensor(out=r16[:, :S, :], in0=r32[:, :S, 0:16],
                                        in1=r32[:, :S, 16:32], op=Alu.add)
                e = p2.tile([P, S_max, H], f32, tag="e")
                nc.vector.tensor_tensor(out=e[:, :S, :], in0=r16[:, :S, 0:8],
                                        in1=r16[:, :S, 8:16], op=Alu.add)
                ex = p2.tile([P, S_max, H], bf16, tag="ex")
                nc.scalar.activation(out=ex[:, :S, :], in_=e[:, :S, :],
                                     func=Act.Exp)
                nc.vector.tensor_tensor(
                    out=wt[:, :S, 0:OUT].rearrange(
                        "p s (d h) -> p s d h", h=H),
                    in0=xl_t[:, :S, :].rearrange("p s (d h) -> p s d h", h=H),
                    in1=ex[:, :S, None, :].to_broadcast((P, S, D, H)),
                    op=Alu.mult,
                )
                nc.scalar.copy(out=wt[:, :S, OUT : OUT + H], in_=ex[:, :S, :])

            def gp_slice(gp_banks, gi):
                return gp_banks[gi // 3][:, gi % 3, :]

            def do_epilogue(s, groups, gp_banks):
                nw = len(groups)
                g0 = groups[0]
                n_banks = math.ceil(nw / 3)
                stage = ep.tile([P, G_SPAN, 136], bf16, tag="stage")
                for b in range(n_banks):
                    hi = min(3, nw - 3 * b)
                    nc.scalar.copy(out=stage[:, 3 * b : 3 * b + hi, :],
                                   in_=gp_banks[b][:, :hi, :])
                num = stage[:, :nw, 0:OUT]
                den_c = ep.tile([P, G_SPAN, H], f32, tag="denc")
                nc.scalar.copy(out=den_c[:, :nw, :],
                               in_=stage[:, :nw, OUT : OUT + H])
                rd = ep.tile([P, G_SPAN, H], f32, tag="rd")
                nc.vector.tensor_scalar_add(rd[:, :nw, :], den_c[:, :nw, :],
                                            DEN_EPS)
                nc.vector.reciprocal(rd[:, :nw, :], rd[:, :nw, :])
                o1 = ep.tile([P, G_SPAN, OUT], bf16, tag="o1")
                nc.vector.tensor_tensor(
                    out=o1[:, :nw, :].rearrange("p j (d h) -> p j d h", h=H),
                    in0=num.rearrange("p j (d h) -> p j d h", h=H),
                    in1=rd[:, :nw, None, :].to_broadcast((P, nw, D, H)),
                    op=Alu.mult,
                )
                xres = ep.tile([P, G_SPAN, OUT], bf16, tag="xres")
                nc.sync.dma_start(
                    out=xres[:, :nw, :],
                    in_=xres_d[g0 * P : (g0 + nw) * P, :].rearrange(
                        "(j p) f -> p j f", p=P
                    ),
                )
                vmin = ep.tile([P, G_SPAN, OUT], bf16, tag="vmin")
                nc.vector.tensor_scalar_min(vmin[:, :nw, :], o1[:, :nw, :], 0.0)
                ev = ep.tile([P, G_SPAN, OUT], bf16, tag="ev")
                nc.scalar.activation(out=ev[:, :nw, :], in_=vmin[:, :nw, :],
                                     func=Act.Exp)
                t1 = ep.tile([P, G_SPAN, OUT], bf16, tag="t1")
                nc.vector.scalar_tensor_tensor(
                    out=t1[:, :nw, :], in0=o1[:, :nw, :], scalar=0.0,
                    in1=ev[:, :nw, :], op0=Alu.max, op1=Alu.add,
                )
                v = ep.tile([P, G_SPAN, OUT], bf16, tag="v")
                nc.vector.scalar_tensor_tensor(
                    out=v[:, :nw, :], in0=xres[:, :nw, :], scalar=-1.0,
                    in1=t1[:, :nw, :], op0=Alu.add, op1=Alu.add,
                )
                # LayerNorm (gamma=1, beta=0)
                mu = ep.tile([P, G_SPAN], f32, tag="mu")
                nc.vector.tensor_reduce(out=mu[:, :nw], in_=v[:, :nw, :],
                                        axis=mybir.AxisListType.X, op=Alu.add)
                mu2 = ep.tile([P, G_SPAN], f32, tag="mu2")
                nc.vector.tensor_scalar_mul(mu2[:, :nw], mu[:, :nw], 1.0 / OUT)
                cen = ep.tile([P, G_SPAN, OUT], bf16, tag="cen")
                nc.vector.tensor_tensor(
                    out=cen[:, :nw, :], in0=v[:, :nw, :],
                    in1=mu2[:, :nw, None].to_broadcast((P, nw, OUT)),
                    op=Alu.subtract,
                )
                sq = ep.tile([P, G_SPAN, OUT], bf16, tag="sq")
                nc.vector.tensor_tensor(out=sq[:, :nw, :], in0=cen[:, :nw, :],
                                        in1=cen[:, :nw, :], op=Alu.mult)
                var0 = ep.tile([P, G_SPAN], f32, tag="var0")
                nc.vector.tensor_reduce(out=var0[:, :nw], in_=sq[:, :nw, :],
                                        axis=mybir.AxisListType.X, op=Alu.add)
                var1 = ep.tile([P, G_SPAN], f32, tag="var1")
                nc.vector.tensor_scalar(
                    out=var1[:, :nw], in0=var0[:, :nw],
                    scalar1=1.0 / OUT, scalar2=LN_EPS,
                    op0=Alu.mult, op1=Alu.add,
                )
                var = ep.tile([P, G_SPAN], f32, tag="var")
                nc.scalar.activation(out=var[:, :nw], in_=var1[:, :nw],
                                     func=Act.Sqrt)
                nc.vector.reciprocal(var[:, :nw], var[:, :nw])
                o2 = ep.tile([P, G_SPAN, OUT], f32, tag="o2")
                # output AP un-permutes d-major back to h-major
                nc.vector.tensor_tensor(
                    out=o2[:, :nw, :].rearrange("p j (h d) -> p j d h", h=H),
                    in0=cen[:, :nw, :].rearrange("p j (d h) -> p j d h", h=H),
                    in1=var[:, :nw, None, None].to_broadcast((P, nw, D, H)),
                    op=Alu.mult,
                )
                glast = groups[-1]
                full = nw - 1 if glast == n_groups - 1 else nw
                if full:
                    nc.scalar.dma_start(
                        out=out_own[g0 * P : (g0 + full) * P, :].rearrange(
                            "(j p) f -> p j f", p=P
                        ),
                        in_=o2[:, :full, :],
                    )
                if full != nw:
                    nc.scalar.dma_start(
                        out=out_own[glast * P : glast * P + last_rows, :],
                        in_=o2[:last_rows, nw - 1, :],
                    )


            pending = None

            for s in range(n_spans):
                groups = list(range(s * G_SPAN, min((s + 1) * G_SPAN,
                                                    n_groups)))
                nw = len(groups)
                g0 = groups[0]
                n_banks = math.ceil(nw / 3)
                gp_banks = [
                    gps.tile([P, 3, 136], f32, tag=f"gp{b}", name=f"gp{b}")
                    for b in range(n_banks)
                ]

                # ---- self-loop pseudo-segment (dense, identity selection) --
                wt_self = p2.tile([P, G_SPAN, 136], bf16, tag="wts")
                xlo = p2.tile([P, G_SPAN, OUT], bf16, tag="xlo")
                nc.sync.dma_start(
                    out=xlo[:, :nw, :],
                    in_=xlo_tab[g0 * P : (g0 + nw) * P, :].rearrange(
                        "(j p) f -> p j f", p=P
                    ),
                )

                def m_self(c0, c1, _xlo=xlo, _g0=g0):
                    mp = xps.tile([P, 4, OUT], f32, tag="m", bufs=2)
                    for j in range(c0, c1):
                        nc.tensor.matmul(
                            out=mp[:, j - c0, :], lhsT=id8_s[:],
                            rhs=xrs_all[:, _g0 + j, :],
                            start=True, stop=False,
                        )
                        nc.tensor.matmul(
                            out=mp[:, j - c0, :], lhsT=id8_s[:],
                            rhs=_xlo[:, j, :],
                            start=False, stop=True,
                        )
                    return mp[:, : c1 - c0, :]

                seg_compute(nw, xlo[:, :nw, :], m_self, wt_self)
                # one wide start per PSUM bank: the start bit zeroes the
                # whole bank, so each bank must be opened by a single matmul
                # covering all of its group slices
                for b in range(n_banks):
                    hi = min(3, nw - 3 * b)
                    nc.tensor.matmul(
                        out=gp_banks[b][:, :hi, :].rearrange(
                            "p j f -> p (j f)"),
                        lhsT=id8_s[:],
                        rhs=wt_self[:, 3 * b : 3 * b + hi, :].rearrange(
                            "p j f -> p (j f)"),
                        start=True,
                        stop=bank_stop[(s, b)] is None,
                    )

                # ---- per-quartile gathered segments ------------------------
                for q in range(NQ):
                    S = int(seg_sub[s, q])
                    io = int(seg_idx_off[s, q])
                    so = int(seg_sel_off[s, q])
                    em = emits[(s, q)]
                    idx_t = p2.tile([P, 8 * S_max], i16, tag="idx", bufs=3)
                    nc.sync.dma_start(out=idx_t[:, : 8 * S],
                                      in_=idx_d[:, io : io + 8 * S])
                    xl_e = p2.tile([P, S_max, OUT], bf16, tag="xl", bufs=3)
                    # chunked to <=1024 rows per call (the HW descriptor
                    # ring rejects much larger single calls); queues
                    # round-robin so DGE runs on all four Q7 core pairs
                    for j0 in range(0, S, 8):
                        j1 = min(j0 + 8, S)
                        nn = (j1 - j0) * P
                        nc.gpsimd.dma_gather(
                            out_ap=xl_e[:, j0:j1, :],
                            in_ap=xl_tab_q[q][:],
                            idxs_ap=idx_t[:, 8 * j0 : 8 * j1],
                            num_idxs=nn,
                            num_idxs_reg=nn,
                            elem_size=OUT,
                            queue_num=gather_rr[0] % N_QUEUES,
                        )
                        gather_rr[0] += 1
                    selb = p2.tile([P, em_max, P], fp8, tag="sel", bufs=2)
                    nc.sync.dma_start(
                        out=selb[:, : len(em), :].rearrange("p i d -> p (i d)"),
                        in_=sel_d[:, so : so + len(em) * P],
                    )
                    selTb = p2.tile([P, em_max, P], fp8, tag="selT", bufs=2)
                    nc.sync.dma_start(
                        out=selTb[:, : len(em), :].rearrange(
                            "p i d -> p (i d)"),
                        in_=selT_d[:, so : so + len(em) * P],
                    )

                    sub_windows = {}
                    for i, (j, g) in enumerate(em):
                        sub_windows.setdefault(j, []).append((i, g))

                    def m_seg(c0, c1, _sw=sub_windows, _selT=selTb,
                              _xl=xl_e, _g0=g0):
                        mp = xps.tile([P, 4, OUT], f32, tag="m", bufs=2)
                        for j in range(c0, c1):
                            for wi, (i, g) in enumerate(_sw[j]):
                                nc.tensor.matmul(
                                    out=mp[:, j - c0, :],
                                    lhsT=_selT[:, i, :],
                                    rhs=xrs_all[:, g, :],
                                    start=wi == 0, stop=False,
                                )
                            nc.tensor.matmul(
                                out=mp[:, j - c0, :], lhsT=id8_s[:],
                                rhs=_xl[:, j, :],
                                start=False, stop=True,
                            )
                        return mp[:, : c1 - c0, :]

                    wt = p2.tile([P, S_max, 136], bf16, tag="wt")
                    seg_compute(S, xl_e, m_seg, wt)
                    for i, (j, g) in enumerate(em):
                        nc.tensor.matmul(
                            out=gp_slice(gp_banks, g - g0),
                            lhsT=selb[:, i, :],
                            rhs=wt[:, j, :],
                            start=False,
                            stop=bank_stop[(s, (g - g0) // 3)] == (q, i),
                        )

                # epilogue pipelined one span behind (gp bufs=2)
                if pending is not None:
                    do_epilogue(*pending)
                pending = (s, groups, gp_banks)

            do_epilogue(*pending)

    nc.finalize()
    return nc


# ---------------------------------------------------------------------------
# Host entry point
# ---------------------------------------------------------------------------

TRACE = False       # set by test harness to collect an NTFF profile
LAST = {}           # stash of the last BassKernelResults (for test.py)


def kernel(x, edge_index, W_l, b_l, W_r, b_r, att, bias, gamma, beta):
    from concourse.bass_utils import run_bass_kernel_spmd

    x = np.asarray(x, dtype=np.float32)
    edge_index = np.asarray(edge_index)
    n_nodes = x.shape[0]
    n_cores = N_CORES

    assert not np.any(b_l) and not np.any(b_r) and not np.any(bias)
    assert not np.any(np.asarray(gamma) != 1.0) and not np.any(beta)

    sched, idx_arrays, sel_arrays, selT_arrays, per = _preprocess(
        edge_index, n_nodes, n_cores
    )
    n_groups = sched["n_groups"]
    own_pad = n_groups * P

    nc = _build_program(n_nodes, per, sched)

    qs = n_nodes // NQ
    n_blk_q = math.ceil(qs / 1024)
    x_pad = (NQ - 1) * qs + n_blk_q * 1024
    n_blk_own = math.ceil(own_pad / 1024)
    own_x_pad = n_blk_own * 1024

    x16 = np.zeros((x_pad, IN), dtype=BF16)
    x16[:n_nodes] = x.astype(BF16)
    x16t = np.ascontiguousarray(x16.T)
    wl16 = np.asarray(W_l, np.float32)[:, PERM].astype(BF16)
    wr16 = np.asarray(W_r, np.float32)[:, PERM].astype(BF16)
    att_p = np.asarray(att, np.float32).reshape(OUT)[PERM]
    att16 = np.tile(att_p[None, :], (P, 1)).astype(BF16)
    ident8 = np.eye(P, dtype=np.float32).astype(FP8)

    in_maps = []
    for c in range(n_cores):
        xo16 = np.zeros((own_x_pad, IN), dtype=BF16)
        xo16[:per] = x16[c * per : (c + 1) * per]
        xo16t = np.ascontiguousarray(xo16.T)
        xres16 = np.zeros((own_pad, IN), dtype=BF16)
        xres16[:per] = x[c * per : (c + 1) * per, PERM].astype(BF16)
        in_maps.append({
            "x16t": x16t,
            "xo16t": xo16t,
            "wl16": wl16,
            "wr16": wr16,
            "att16": att16,
            "ident8": ident8,
            "idx": idx_arrays[c],
            "sel8": sel_arrays[c],
            "selT8": selT_arrays[c],
            "xres16": xres16,
        })

    res = run_bass_kernel_spmd(nc, in_maps, list(range(n_cores)), trace=TRACE)
    LAST["res"] = res
    outs = [res.results[c]["out_own"][:per] for c in range(n_cores)]
    return np.concatenate(outs, axis=0).astype(np.float32)


# revision 20
# speedup vs baseline: 1.0765x; 1.0765x over previous
"""GATv2 layer (100k nodes, 800k edges + self-loops, 8 heads x 16 dim) on 8 TRN2
cores — v2.

Destination nodes are partitioned across the 8 cores (12.5k each).  Real edges
are bucketed per (dst-group-of-128, src-quartile) cell with a shared SPMD
schedule; self-loops run as a dense per-span pseudo-segment with identity
selection (they open each group's PSUM accumulators).

Key structure (v2):
- Gather descriptor generation is the scarce resource (~8 ns/row of Q7 time),
  so gathers are issued one-per-segment and round-robined over 4 SWDGE queues:
  each queue's descriptors are generated by a different Q7 core pair.
- One-hot selection matrices (sel = emission lhsT, selT = xr-selection lhsT)
  are precomputed on the host in fp8 (0/1 exact) and streamed from HBM —
  no on-device is_equal / transpose / PSUM copies.
- m = xr[dst] + xl[src] is accumulated on the PE: selT-matmul from the
  span-resident xr table plus an identity-matmul of the gathered xl rows.
  LeakyReLU runs on the scalar engine straight out of PSUM.
- Features use a d-major permutation (col d*8+h holds head h, dim d), baked
  into W_l/W_r/att/residual on the host: the per-head exp() broadcast multiply
  and the attention-dot reduction (a pure halving tree) then run at the DVE's
  2x bf16 rate.  The final LayerNorm op un-permutes via its output AP.
"""

import math

import numpy as np
import ml_dtypes

P = 128
H, D = 8, 16
IN = 128
OUT = 128
NEG_SLOPE = 0.2
LN_EPS = 1e-5
DEN_EPS = 1e-16

N_CORES = 8
NQ = 4            # src quartiles (int16 gather index range)
G_SPAN = 8        # dst groups per span (3 PSUM emission banks per span)
N_QUEUES = 4      # SWDGE queues for gather DGE parallelism

BF16 = ml_dtypes.bfloat16
FP8 = ml_dtypes.float8_e4m3

# permutation: permuted column d*8+h holds original column h*16+d
PERM = np.array([h * D + d for d in range(D) for h in range(H)], dtype=np.int64)


# ---------------------------------------------------------------------------
# CPU preprocessing: cell bucketing + static SPMD schedule + one-hot blobs
# ---------------------------------------------------------------------------

def _preprocess(edge_index: np.ndarray, n_nodes: int, n_cores: int):
    src = edge_index[0].astype(np.int64)
    dst = edge_index[1].astype(np.int64)

    assert n_nodes % n_cores == 0
    per = n_nodes // n_cores
    qs = n_nodes // NQ
    n_groups = math.ceil(per / P)
    n_spans = math.ceil(n_groups / G_SPAN)

    core = dst // per
    g_loc = (dst - core * per) // P
    quart = src // qs
    span = g_loc // G_SPAN

    key = (((core * n_spans + span) * NQ + quart) * n_groups + g_loc) * np.int64(
        n_nodes
    ) + src
    order = np.argsort(key, kind="stable")
    src = src[order]
    dst = dst[order]
    core = core[order]
    g_loc = g_loc[order]
    quart = quart[order]

    # shared cell caps (32-multiples, >=128 so subtiles span <=2 groups)
    cell_key = (core * n_groups + g_loc) * NQ + quart
    cnt = np.bincount(cell_key, minlength=n_cores * n_groups * NQ).reshape(
        n_cores, n_groups, NQ
    )
    cap = np.maximum(P, ((cnt.max(axis=0) + 31) // 32) * 32)  # [n_groups, NQ]

    seg_sub = np.zeros((n_spans, NQ), dtype=np.int64)
    cell_off = np.zeros((n_groups, NQ), dtype=np.int64)
    emits = {}        # (s,q) -> [(j, g)] in emission order
    for s in range(n_spans):
        gs = list(range(s * G_SPAN, min((s + 1) * G_SPAN, n_groups)))
        for q in range(NQ):
            off = 0
            bounds = []
            for g in gs:
                cell_off[g, q] = off
                bounds.append((off, g))
                off += cap[g, q]
            n_sub = (off + P - 1) // P
            seg_sub[s, q] = n_sub
            em = []
            for j in range(n_sub):
                lo, hi = j * P, min((j + 1) * P, off)
                cells = [g for (st, g) in bounds
                         if st < hi and st + cap[g, q] > lo]
                assert 1 <= len(cells) <= 2, (s, q, j, cells)
                for g in cells:
                    em.append((j, g))
            emits[(s, q)] = em
    S_max = int(seg_sub.max())
    assert S_max * P <= 2944, "gather exceeds SWDGE ring"

    # last touch per PSUM bank (3 groups each) across the span's real-edge
    # emission streams; None if the bank only sees its self-segment opener
    bank_stop = {}
    for s in range(n_spans):
        gs = list(range(s * G_SPAN, min((s + 1) * G_SPAN, n_groups)))
        n_banks = math.ceil(len(gs) / 3)
        for b in range(n_banks):
            bank_stop[(s, b)] = None
        for q in range(NQ):
            for i, (j, g) in enumerate(emits[(s, q)]):
                bank_stop[(s, (g - gs[0]) // 3)] = (q, i)

    # column layouts
    seg_idx_off = np.zeros((n_spans, NQ), dtype=np.int64)   # idx cols (8S each)
    seg_sel_off = np.zeros((n_spans, NQ), dtype=np.int64)   # sel cols (128*em)
    seg_em = np.zeros((n_spans, NQ), dtype=np.int64)
    c_idx = 0
    c_sel = 0
    for s in range(n_spans):
        for q in range(NQ):
            seg_idx_off[s, q] = c_idx
            c_idx += 8 * seg_sub[s, q]
            seg_sel_off[s, q] = c_sel
            seg_em[s, q] = len(emits[(s, q)])
            c_sel += P * seg_em[s, q]

    # per-segment slot base in the flat slot vector
    seg_slot_off = np.zeros((n_spans, NQ), dtype=np.int64)
    t = 0
    for s in range(n_spans):
        for q in range(NQ):
            seg_slot_off[s, q] = t
            t += seg_sub[s, q] * P
    total_slots = t

    em_max = int(seg_em.max())

    idx_arrays = []
    sel_arrays = []
    selT_arrays = []
    for c in range(n_cores):
        m = core == c
        e_src = src[m]
        e_dst = dst[m]
        e_g = g_loc[m]
        e_q = quart[m]
        e_span = e_g // G_SPAN
        ck = (e_span * NQ + e_q) * n_groups + e_g
        changes = np.ones(len(ck), dtype=bool)
        changes[1:] = ck[1:] != ck[:-1]
        starts = np.flatnonzero(changes)
        rank = np.arange(len(ck)) - np.repeat(starts, np.diff(
            np.append(starts, len(ck))))
        slot = (seg_slot_off[e_span, e_q] + cell_off[e_g, e_q] + rank)

        xl_idx = np.zeros(total_slots, dtype=np.int16)
        dloc = np.full(total_slots, -1, dtype=np.int64)   # dst local row
        # table row of node n (quartile-local): block base + (n%1024
        # permuted): stored row = base + (n % 128) * 8 + (n % 1024) // 128
        ql = e_src - e_q * qs
        xl_idx[slot] = ((ql // 1024) * 1024 + (ql % P) * 8
                        + (ql % 1024) // P).astype(np.int16)
        dloc[slot] = e_dst - c * per

        packed = np.zeros((P, c_idx), dtype=np.int16)
        sel8 = np.zeros((P, c_sel), dtype=FP8)
        selT8 = np.zeros((P, c_sel), dtype=FP8)
        for s in range(n_spans):
            for q in range(NQ):
                S = int(seg_sub[s, q])
                if S == 0:
                    continue
                o = int(seg_slot_off[s, q])
                n = S * P
                co = int(seg_idx_off[s, q])
                packed[:, co : co + 8 * S] = np.tile(
                    xl_idx[o : o + n].reshape(-1, 16).T, (8, 1)
                )
                so = int(seg_sel_off[s, q])
                dl = dloc[o : o + n].reshape(S, P)    # [subtile, slot]
                for i, (j, g) in enumerate(emits[(s, q)]):
                    hot = dl[j] - g * P               # [P] values or <0
                    ok = (hot >= 0) & (hot < P)
                    rows = np.flatnonzero(ok)
                    one = np.zeros((P, P), dtype=FP8)
                    one[rows, hot[rows]] = 1.0
                    sel8[:, so + i * P : so + (i + 1) * P] = one
                    selT8[:, so + i * P : so + (i + 1) * P] = one.T
        idx_arrays.append(packed)
        sel_arrays.append(sel8)
        selT_arrays.append(selT8)

    sched = {
        "n_groups": n_groups,
        "n_spans": n_spans,
        "seg_sub": seg_sub,
        "seg_idx_off": seg_idx_off,
        "seg_sel_off": seg_sel_off,
        "seg_em": seg_em,
        "emits": emits,
        "bank_stop": bank_stop,
        "c_idx": c_idx,
        "c_sel": c_sel,
        "S_max": S_max,
        "em_max": em_max,
    }
    return sched, idx_arrays, sel_arrays, selT_arrays, per


# ---------------------------------------------------------------------------
# Bass program (shared by all cores)
# ---------------------------------------------------------------------------

def _build_program(n_nodes, per, sched):
    from contextlib import ExitStack

    from concourse import bass, mybir
    from concourse import tile as tile_mod
    from concourse.bacc import Bacc

    f32 = mybir.dt.float32
    bf16 = mybir.dt.bfloat16
    fp8 = mybir.dt.float8e4
    i16 = mybir.dt.int16
    Alu = mybir.AluOpType
    Act = mybir.ActivationFunctionType

    qs = n_nodes // NQ
    n_groups = sched["n_groups"]
    n_spans = sched["n_spans"]
    seg_sub = sched["seg_sub"]
    seg_idx_off = sched["seg_idx_off"]
    seg_sel_off = sched["seg_sel_off"]
    emits = sched["emits"]
    bank_stop = sched["bank_stop"]
    c_idx = sched["c_idx"]
    c_sel = sched["c_sel"]
    S_max = sched["S_max"]
    em_max = sched["em_max"]
    own_pad = n_groups * P
    last_rows = per - (n_groups - 1) * P

    n_blk_q = math.ceil((n_nodes // NQ) / 1024)   # phase-1 blocks per quartile
    q_pad = n_blk_q * 1024
    x_pad = (NQ - 1) * qs + q_pad
    n_blk_own = math.ceil(own_pad / 1024)
    own_x_pad = n_blk_own * 1024

    nc = Bacc(dynamic_dma_scratch_size=32768, num_swdge_queues=N_QUEUES)

    x16t = nc.declare_dram_parameter("x16t", [IN, x_pad], bf16,
                                     isOutput=False)
    xo16t = nc.declare_dram_parameter("xo16t", [IN, own_x_pad], bf16,
                                      isOutput=False)
    wl_d = nc.declare_dram_parameter("wl16", [IN, OUT], bf16, isOutput=False)
    wr_d = nc.declare_dram_parameter("wr16", [IN, OUT], bf16, isOutput=False)
    att_d = nc.declare_dram_parameter("att16", [P, OUT], bf16, isOutput=False)
    id8_d = nc.declare_dram_parameter("ident8", [P, P], fp8, isOutput=False)
    idx_d = nc.declare_dram_parameter("idx", [P, c_idx], i16, isOutput=False)
    sel_d = nc.declare_dram_parameter("sel8", [P, c_sel], fp8, isOutput=False)
    selT_d = nc.declare_dram_parameter("selT8", [P, c_sel], fp8,
                                       isOutput=False)
    xres_d = nc.declare_dram_parameter("xres16", [own_pad, IN], bf16,
                                       isOutput=False)
    out_own = nc.declare_dram_parameter("out_own", [own_pad, OUT], f32,
                                        isOutput=True)

    xl_tab_q = [nc.dram_tensor(f"xl_tab_q{q}", [q_pad, OUT], bf16)
                for q in range(NQ)]
    xlo_tab = nc.dram_tensor("xlo_tab", [own_pad, OUT], bf16)

    with tile_mod.TileContext(nc) as tc, ExitStack() as ctx:
        const = ctx.enter_context(tc.tile_pool(name="const", bufs=1))
        wl_s = const.tile([IN, OUT], bf16)
        wr_s = const.tile([IN, OUT], bf16)
        att_s = const.tile([P, OUT], bf16)
        id8_s = const.tile([P, P], fp8)
        xrs_all = const.tile([P, n_groups, OUT], bf16)   # own xr, resident
        nc.sync.dma_start(out=wl_s[:], in_=wl_d[:])
        nc.sync.dma_start(out=wr_s[:], in_=wr_d[:])
        nc.sync.dma_start(out=att_s[:], in_=att_d[:])
        nc.sync.dma_start(out=id8_s[:], in_=id8_d[:])

        # PE warmup: observe DMA-loaded weights once.
        with tc.tile_pool(name="warm", bufs=1, space="PSUM") as warm:
            warm_p = warm.tile([P, OUT], f32)
            nc.tensor.matmul(out=warm_p[:], lhsT=wl_s[:], rhs=wl_s[:],
                             start=True, stop=True)
            nc.tensor.matmul(out=warm_p[:], lhsT=wr_s[:], rhs=wr_s[:],
                             start=True, stop=True)
            nc.tensor.matmul(out=warm_p[:], lhsT=id8_s[:], rhs=wl_s[:],
                             start=True, stop=True)

        # ------------------------------------------------------------------
        # Phase 1a: own rows -> resident xr / xl tables (no HBM roundtrip)
        # ------------------------------------------------------------------
        with tc.tile_pool(name="p1o", bufs=3) as p1o, \
             tc.tile_pool(name="p1ops", bufs=2, space="PSUM") as p1ops:
            for b in range(n_blk_own):
                r0 = b * 1024
                xt = p1o.tile([P, 1024], bf16, tag="xt")
                nc.sync.dma_start(out=xt[:], in_=xo16t[:, r0 : r0 + 1024])
                g0 = r0 // P
                n_j = min(8, n_groups - g0)
                # xr: into the resident SBUF table
                o_p = p1ops.tile([P, 1024], f32, tag="opr")
                for j in range(n_j):
                    nc.tensor.matmul(
                        out=o_p[:, j * P : (j + 1) * P],
                        lhsT=xt[:, j * P : (j + 1) * P],
                        rhs=wr_s[:],
                        start=True,
                        stop=True,
                    )
                nc.scalar.copy(
                    out=xrs_all[:, g0 : g0 + n_j, :].rearrange(
                        "p j f -> p (j f)"
                    ),
                    in_=o_p[:, : n_j * P],
                )
                # xl of own rows: staged to DRAM (per-span self segments)
                o_p2 = p1ops.tile([P, 1024], f32, tag="opl")
                for j in range(n_j):
                    nc.tensor.matmul(
                        out=o_p2[:, j * P : (j + 1) * P],
                        lhsT=xt[:, j * P : (j + 1) * P],
                        rhs=wl_s[:],
                        start=True,
                        stop=True,
                    )
                stg = p1o.tile([P, 8, OUT], bf16, tag="stg")
                nc.scalar.copy(
                    out=stg[:, :n_j, :].rearrange("p j f -> p (j f)"),
                    in_=o_p2[:, : n_j * P],
                )
                nc.scalar.dma_start(
                    out=xlo_tab[r0 : r0 + n_j * P, :].rearrange(
                        "(j p) f -> p j f", p=P
                    ),
                    in_=stg[:, :n_j, :],
                )

        # ------------------------------------------------------------------
        # Phase 1b: xl projection tables (gather sources), one per quartile
        # so phase-2 segments for quartile q only wait on q's table
        # ------------------------------------------------------------------
        with tc.tile_pool(name="p1", bufs=6) as p1, \
             tc.tile_pool(name="p1ps", bufs=4, space="PSUM") as p1ps:
            for q in range(NQ):
                for b in range(n_blk_q):
                    r0 = q * qs + b * 1024
                    xt = p1.tile([P, 1024], bf16, tag="xt")
                    nc.sync.dma_start(out=xt[:], in_=x16t[:, r0 : r0 + 1024])
                    o_p = p1ps.tile([P, 1024], f32, tag="op")
                    for j in range(8):
                        nc.tensor.matmul(
                            out=o_p[:, j * P : (j + 1) * P],
                            lhsT=xt[:, j * P : (j + 1) * P],
                            rhs=wl_s[:],
                            start=True,
                            stop=True,
                        )
                    stg = p1.tile([P, 8, OUT], bf16, tag="stg")
                    if b % 2 == 0:
                        nc.scalar.copy(
                            out=stg[:].rearrange("p j f -> p (j f)"),
                            in_=o_p[:],
                        )
                    else:
                        nc.vector.tensor_copy(
                            out=stg[:].rearrange("p j f -> p (j f)"),
                            in_=o_p[:],
                        )
                    # permuted row order: block row p*8+j holds node j*128+p,
                    # making each partition's store one contiguous 2KB run
                    # (gather indices are host-permuted to match)
                    nc.sync.dma_start(
                        out=xl_tab_q[q][b * 1024 : (b + 1) * 1024, :].rearrange(
                            "(p j) f -> p j f", j=8
                        ),
                        in_=stg[:],
                    )

        # ------------------------------------------------------------------
        # Phase 2
        # ------------------------------------------------------------------
        with tc.tile_pool(name="p2", bufs=2) as p2, \
             tc.tile_pool(name="gps", bufs=2, space="PSUM") as gps, \
             tc.tile_pool(name="xps", bufs=2, space="PSUM") as xps, \
             tc.tile_pool(name="ep", bufs=2) as ep:

            att_bc = att_s[:][:, None, :]
            gather_rr = [0]

            def seg_compute(S, xl_t, m_src, wt):
                """lrelu -> att-dot tree -> exp -> weighted values.

                m_src(c0, c1) yields the PSUM AP holding m for subtiles
                [c0, c1); xl_t is the SBUF bf16 xl rows tile [P, S, OUT].
                Returns the wt tile filled in [:, :S, :136].
                """
                t = p2.tile([P, S_max, OUT], bf16, tag="t")
                for c0 in range(0, S, 4):
                    c1 = min(c0 + 4, S)
                    nc.scalar.activation(
                        out=t[:, c0:c1, :], in_=m_src(c0, c1),
                        func=Act.Prelu, alpha=NEG_SLOPE,
                    )
                u = p2.tile([P, S_max, OUT], bf16, tag="u")
                nc.vector.tensor_tensor(
                    out=u[:, :S, :], in0=t[:, :S, :],
                    in1=att_bc.to_broadcast((P, S, OUT)), op=Alu.mult,
                )
                # halving tree over d (d-major layout: col = d*8 + h)
                r64 = p2.tile([P, S_max, 64], bf16, tag="r64")
                nc.vector.tensor_tensor(out=r64[:, :S, :], in0=u[:, :S, 0:64],
                                        in1=u[:, :S, 64:128], op=Alu.add)
                r32 = p2.tile([P, S_max, 32], bf16, tag="r32")
                nc.vector.tensor_tensor(out=r32[:, :S, :], in0=r64[:, :S, 0:32],
                                        in1=r64[:, :S, 32:64], op=Alu.add)
                r16 = p2.tile([P, S_max, 16], bf16, tag="r16")
                nc.vector.tensor_tensor(out=r16[:, :S, :], in0=r32[:, :S, 0:16],
                                        in1=r32[:, :S, 16:32], op=Alu.add)
                e = p2.tile([P, S_max, H], f32, tag="e")
                nc.vector.tensor_tensor(out=e[:, :S, :], in0=r16[:, :S, 0:8],
                                        in1=r16[:, :S, 8:16], op=Alu.add)
                ex = p2.tile([P, S_max, H], bf16, tag="ex")
                nc.scalar.activation(out=ex[:, :S, :], in_=e[:, :S, :],
                                     func=Act.Exp)
                nc.vector.tensor_tensor(
                    out=wt[:, :S, 0:OUT].rearrange(
                        "p s (d h) -> p s d h", h=H),
                    in0=xl_t[:, :S, :].rearrange("p s (d h) -> p s d h", h=H),
                    in1=ex[:, :S, None, :].to_broadcast((P, S, D, H)),
                    op=Alu.mult,
                )
                nc.scalar.copy(out=wt[:, :S, OUT : OUT + H], in_=ex[:, :S, :])

            def gp_slice(gp_banks, gi):
                return gp_banks[gi // 3][:, gi % 3, :]

            def do_epilogue(s, groups, gp_banks):
                nw = len(groups)
                g0 = groups[0]
                n_banks = math.ceil(nw / 3)
                stage = ep.tile([P, G_SPAN, 136], bf16, tag="stage")
                for b in range(n_banks):
                    hi = min(3, nw - 3 * b)
                    nc.scalar.copy(out=stage[:, 3 * b : 3 * b + hi, :],
                                   in_=gp_banks[b][:, :hi, :])
                num = stage[:, :nw, 0:OUT]
                den_c = ep.tile([P, G_SPAN, H], f32, tag="denc")
                nc.scalar.copy(out=den_c[:, :nw, :],
                               in_=stage[:, :nw, OUT : OUT + H])
                rd = ep.tile([P, G_SPAN, H], f32, tag="rd")
                nc.vector.tensor_scalar_add(rd[:, :nw, :], den_c[:, :nw, :],
                                            DEN_EPS)
                nc.vector.reciprocal(rd[:, :nw, :], rd[:, :nw, :])
                o1 = ep.tile([P, G_SPAN, OUT], bf16, tag="o1")
                nc.vector.tensor_tensor(
                    out=o1[:, :nw, :].rearrange("p j (d h) -> p j d h", h=H),
                    in0=num.rearrange("p j (d h) -> p j d h", h=H),
                    in1=rd[:, :nw, None, :].to_broadcast((P, nw, D, H)),
                    op=Alu.mult,
                )
                xres = ep.tile([P, G_SPAN, OUT], bf16, tag="xres")
                nc.sync.dma_start(
                    out=xres[:, :nw, :],
                    in_=xres_d[g0 * P : (g0 + nw) * P, :].rearrange(
                        "(j p) f -> p j f", p=P
                    ),
                )
                vmin = ep.tile([P, G_SPAN, OUT], bf16, tag="vmin")
                nc.vector.tensor_scalar_min(vmin[:, :nw, :], o1[:, :nw, :], 0.0)
                ev = ep.tile([P, G_SPAN, OUT], bf16, tag="ev")
                nc.scalar.activation(out=ev[:, :nw, :], in_=vmin[:, :nw, :],
                                     func=Act.Exp)
                t1 = ep.tile([P, G_SPAN, OUT], bf16, tag="t1")
                nc.vector.scalar_tensor_tensor(
                    out=t1[:, :nw, :], in0=o1[:, :nw, :], scalar=0.0,
                    in1=ev[:, :nw, :], op0=Alu.max, op1=Alu.add,
                )
                v = ep.tile([P, G_SPAN, OUT], bf16, tag="v")
                nc.vector.scalar_tensor_tensor(
                    out=v[:, :nw, :], in0=xres[:, :nw, :], scalar=-1.0,
                    in1=t1[:, :nw, :], op0=Alu.add, op1=Alu.add,
                )
                # LayerNorm (gamma=1, beta=0)
                mu = ep.tile([P, G_SPAN], f32, tag="mu")
                nc.vector.tensor_reduce(out=mu[:, :nw], in_=v[:, :nw, :],
                                        axis=mybir.AxisListType.X, op=Alu.add)
                mu2 = ep.tile([P, G_SPAN], f32, tag="mu2")
                nc.vector.tensor_scalar_mul(mu2[:, :nw], mu[:, :nw], 1.0 / OUT)
                cen = ep.tile([P, G_SPAN, OUT], bf16, tag="cen")
                nc.vector.tensor_tensor(
                    out=cen[:, :nw, :], in0=v[:, :nw, :],
                    in1=mu2[:, :nw, None].to_broadcast((P, nw, OUT)),
                    op=Alu.subtract,
                )
                sq = ep.tile([P, G_SPAN, OUT], bf16, tag="sq")
                nc.vector.tensor_tensor(out=sq[:, :nw, :], in0=cen[:, :nw, :],
                                        in1=cen[:, :nw, :], op=Alu.mult)
                var0 = ep.tile([P, G_SPAN], f32, tag="var0")
                nc.vector.tensor_reduce(out=var0[:, :nw], in_=sq[:, :nw, :],
                                        axis=mybir.AxisListType.X, op=Alu.add)
                var1 = ep.tile([P, G_SPAN], f32, tag="var1")
                nc.vector.tensor_scalar(
                    out=var1[:, :nw], in0=var0[:, :nw],
                    scalar1=1.0 / OUT, scalar2=LN_EPS,
                    op0=Alu.mult, op1=Alu.add,
                )
                var = ep.tile([P, G_SPAN], f32, tag="var")
                nc.scalar.activation(out=var[:, :nw], in_=var1[:, :nw],
                                     func=Act.Sqrt)
                nc.vector.reciprocal(var[:, :nw], var[:, :nw])
                o2 = ep.tile([P, G_SPAN, OUT], f32, tag="o2")
                # output AP un-permutes d-major back to h-major
                nc.vector.tensor_tensor(
                    out=o2[:, :nw, :].rearrange("p j (h d) -> p j d h", h=H),
                    in0=cen[:, :nw, :].rearrange("p j (d h) -> p j d h", h=H),
                    in1=var[:, :nw, None, None].to_broadcast((P, nw, D, H)),
                    op=Alu.mult,
                )
                glast = groups[-1]
                full = nw - 1 if glast == n_groups - 1 else nw
                if full:
                    nc.scalar.dma_start(
                        out=out_own[g0 * P : (g0 + full) * P, :].rearrange(
                            "(j p) f -> p j f", p=P
                        ),
                        in_=o2[:, :full, :],
                    )
                if full != nw:
                    nc.scalar.dma_start(
                        out=out_own[glast * P : glast * P + last_rows, :],
                        in_=o2[:last_rows, nw - 1, :],
                    )


            pending = None

            for s in range(n_spans):
                groups = list(range(s * G_SPAN, min((s + 1) * G_SPAN,
                                                    n_groups)))
                nw = len(groups)
                g0 = groups[0]
                n_banks = math.ceil(nw / 3)
                gp_banks = [
                    gps.tile([P, 3, 136], f32, tag=f"gp{b}", name=f"gp{b}")
                    for b in range(n_banks)
                ]

                # ---- self-loop pseudo-segment (dense, identity selection) --
                wt_self = p2.tile([P, G_SPAN, 136], bf16, tag="wts")
                xlo = p2.tile([P, G_SPAN, OUT], bf16, tag="xlo")
                nc.sync.dma_start(
                    out=xlo[:, :nw, :],
                    in_=xlo_tab[g0 * P : (g0 + nw) * P, :].rearrange(
                        "(j p) f -> p j f", p=P
                    ),
                )

                def m_self(c0, c1, _xlo=xlo, _g0=g0):
                    mp = xps.tile([P, 4, OUT], f32, tag="m", bufs=2)
                    for j in range(c0, c1):
                        nc.tensor.matmul(
                            out=mp[:, j - c0, :], lhsT=id8_s[:],
                            rhs=xrs_all[:, _g0 + j, :],
                            start=True, stop=False,
                        )
                        nc.tensor.matmul(
                            out=mp[:, j - c0, :], lhsT=id8_s[:],
                            rhs=_xlo[:, j, :],
                            start=False, stop=True,
                        )
                    return mp[:, : c1 - c0, :]

                seg_compute(nw, xlo[:, :nw, :], m_self, wt_self)
                # one wide start per PSUM bank: the start bit zeroes the
                # whole bank, so each bank must be opened by a single matmul
                # covering all of its group slices
                for b in range(n_banks):
                    hi = min(3, nw - 3 * b)
                    nc.tensor.matmul(
                        out=gp_banks[b][:, :hi, :].rearrange(
                            "p j f -> p (j f)"),
                        lhsT=id8_s[:],
                        rhs=wt_self[:, 3 * b : 3 * b + hi, :].rearrange(
                            "p j f -> p (j f)"),
                        start=True,
                        stop=bank_stop[(s, b)] is None,
                    )

                # ---- per-quartile gathered segments ------------------------
                for q in range(NQ):
                    S = int(seg_sub[s, q])
                    io = int(seg_idx_off[s, q])
                    so = int(seg_sel_off[s, q])
                    em = emits[(s, q)]
                    idx_t = p2.tile([P, 8 * S_max], i16, tag="idx", bufs=3)
                    nc.sync.dma_start(out=idx_t[:, : 8 * S],
                                      in_=idx_d[:, io : io + 8 * S])
                    xl_e = p2.tile([P, S_max, OUT], bf16, tag="xl", bufs=3)
                    # chunked to <=1024 rows per call (the HW descriptor
                    # ring rejects much larger single calls); queues
                    # round-robin so DGE runs on all four Q7 core pairs
                    for j0 in range(0, S, 8):
                        j1 = min(j0 + 8, S)
                        nn = (j1 - j0) * P
                        nc.gpsimd.dma_gather(
                            out_ap=xl_e[:, j0:j1, :],
                            in_ap=xl_tab_q[q][:],
                            idxs_ap=idx_t[:, 8 * j0 : 8 * j1],
                            num_idxs=nn,
                            num_idxs_reg=nn,
                            elem_size=OUT,
                            queue_num=gather_rr[0] % N_QUEUES,
                        )
                        gather_rr[0] += 1
                    selb = p2.tile([P, em_max, P], fp8, tag="sel", bufs=2)
                    nc.sync.dma_start(
                        out=selb[:, : len(em), :].rearrange("p i d -> p (i d)"),
                        in_=sel_d[:, so : so + len(em) * P],
                    )
                    selTb = p2.tile([P, em_max, P], fp8, tag="selT", bufs=2)
                    nc.sync.dma_start(
                        out=selTb[:, : len(em), :].rearrange(
                            "p i d -> p (i d)"),
                        in_=selT_d[:, so : so + len(em) * P],
                    )

                    sub_windows = {}
                    for i, (j, g) in enumerate(em):
                        sub_windows.setdefault(j, []).append((i, g))

                    def m_seg(c0, c1, _sw=sub_windows, _selT=selTb,
                              _xl=xl_e, _g0=g0):
                        mp = xps.tile([P, 4, OUT], f32, tag="m", bufs=2)
                        for j in range(c0, c1):
                            for wi, (i, g) in enumerate(_sw[j]):
                                nc.tensor.matmul(
                                    out=mp[:, j - c0, :],
                                    lhsT=_selT[:, i, :],
                                    rhs=xrs_all[:, g, :],
                                    start=wi == 0, stop=False,
                                )
                            nc.tensor.matmul(
                                out=mp[:, j - c0, :], lhsT=id8_s[:],
                                rhs=_xl[:, j, :],
                                start=False, stop=True,
                            )
                        return mp[:, : c1 - c0, :]

                    wt = p2.tile([P, S_max, 136], bf16, tag="wt")
                    seg_compute(S, xl_e, m_seg, wt)
                    for i, (j, g) in enumerate(em):
                        nc.tensor.matmul(
                            out=gp_slice(gp_banks, g - g0),
                            lhsT=selb[:, i, :],
                            rhs=wt[:, j, :],
                            start=False,
                            stop=bank_stop[(s, (g - g0) // 3)] == (q, i),
                        )

                # epilogue pipelined one span behind (gp bufs=2)
                if pending is not None:
                    do_epilogue(*pending)
                pending = (s, groups, gp_banks)

            do_epilogue(*pending)

    nc.finalize()
    return nc


# ---------------------------------------------------------------------------
# Host entry point
# ---------------------------------------------------------------------------

TRACE = False       # set by test harness to collect an NTFF profile
LAST = {}           # stash of the last BassKernelResults (for test.py)


def kernel(x, edge_index, W_l, b_l, W_r, b_r, att, bias, gamma, beta):
    from concourse.bass_utils import run_bass_kernel_spmd

    x = np.asarray(x, dtype=np.float32)
    edge_index = np.asarray(edge_index)
    n_nodes = x.shape[0]
    n_cores = N_CORES

    assert not np.any(b_l) and not np.any(b_r) and not np.any(bias)
    assert not np.any(np.asarray(gamma) != 1.0) and not np.any(beta)

    sched, idx_arrays, sel_arrays, selT_arrays, per = _preprocess(
        edge_index, n_nodes, n_cores
    )
    n_groups = sched["n_groups"]
    own_pad = n_groups * P

    nc = _build_program(n_nodes, per, sched)

    qs = n_nodes // NQ
    n_blk_q = math.ceil(qs / 1024)
    x_pad = (NQ - 1) * qs + n_blk_q * 1024
    n_blk_own = math.ceil(own_pad / 1024)
    own_x_pad = n_blk_own * 1024

    x16 = np.zeros((x_pad, IN), dtype=BF16)
    x16[:n_nodes] = x.astype(BF16)
    x16t = np.ascontiguousarray(x16.T)
    wl16 = np.asarray(W_l, np.float32)[:, PERM].astype(BF16)
    wr16 = np.asarray(W_r, np.float32)[:, PERM].astype(BF16)
    att_p = np.asarray(att, np.float32).reshape(OUT)[PERM]
    att16 = np.tile(att_p[None, :], (P, 1)).astype(BF16)
    ident8 = np.eye(P, dtype=np.float32).astype(FP8)

    in_maps = []
    for c in range(n_cores):
        xo16 = np.zeros((own_x_pad, IN), dtype=BF16)
        xo16[:per] = x16[c * per : (c + 1) * per]
        xo16t = np.ascontiguousarray(xo16.T)
        xres16 = np.zeros((own_pad, IN), dtype=BF16)
        xres16[:per] = x[c * per : (c + 1) * per, PERM].astype(BF16)
        in_maps.append({
            "x16t": x16t,
            "xo16t": xo16t,
            "wl16": wl16,
            "wr16": wr16,
            "att16": att16,
            "ident8": ident8,
            "idx": idx_arrays[c],
            "sel8": sel_arrays[c],
            "selT8": selT_arrays[c],
            "xres16": xres16,
        })

    res = run_bass_kernel_spmd(nc, in_maps, list(range(n_cores)), trace=TRACE)
    LAST["res"] = res
    outs = [res.results[c]["out_own"][:per] for c in range(n_cores)]
    return np.concatenate(outs, axis=0).astype(np.float32)
